# revision 17
# baseline (speedup 1.0000x reference)
"""Bidirectional-LSTM-cell decoder kernel for 8 Trainium2 NeuronCores.

Strategy (model-parallel over the gate dimension, replicated batch):
  - B=128 lives in the SBUF partition dim everywhere.
  - Each core owns a 128-column slice of every gate (i,f,o,g) of all 4 LSTM
    cells (2 layers x {fwd, rev}) -> 512-wide z slice per cell per core.
  - z = [x | h] @ W^T computed as out[b, gate] = sum_k hT_chunk[k].T @ W_chunk[k]
    with feature-major hT chunks as the stationary operand (fp32, N=512).
  - The embedding lookup of layer 0 is pre-fused on the host:
    Gx0[token] = emb @ W_ih0^T + bias, gathered per step by indirect DMA
    (saves the x-side matmuls entirely).
  - Two AllGathers per step exchange h: feature-major h0 slices (feeds the
    layer-1 x matmuls) and batch-major h1 slices (feeds the fc); the
    feature-major h1 needed by the next step's layer-1 h matmuls is derived
    LOCALLY from the gathered batch-major h1 via 16 PE transposes (this
    removes the third AllGather, whose drain used to stall each step start).
  - fc: logits[i, v] = sum_{q,c} A[c, 16i+q] * fcW[v, 128q+c]  (the reference's
    faithful-to-torch .T.reshape scramble), vocab sharded 8 ways, lhsT = strided
    views A[:, q::16].
  - argmax + softmax-denominator stats are AllGathered (third, tiny AG);
    every core computes the identical label / ended-mask; softmax output is
    vocab-sharded to HBM.
  - Engine-queue hygiene: all

 label-critical-path DMAs go to the Sync queue
    in chain order; the output write goes to the Scalar queue so its blocking
    wait can't head-of-line-delay the stats readback.
  - sigmoid is computed as 0.5*tanh(0.5x)+0.5 (native Sigmoid table is ~30 ULP,
    tanh is ~1.5 ULP); exp only feeds the softmax output.
All matmuls in fp32 (argmax feedback needs fp32-grade logits: the reference
trajectory has top-2 logit gaps down to ~1e-8; 116 of 32768 decisions are
within 1e-5, so any reduced-precision matmul flips labels and diverges).
"""

import sys
import numpy as np

sys.path.insert(0, "/opt/trn_rl_repo")

import concourse.bacc as bacc
import concourse.bass as bass
import concourse.tile as tile
from concourse import mybir
from concourse.bass_utils import run_bass_kernel_spmd

P = 128
NCORES = 8
H = 1024
E = 1024
T_FULL = 256
EOS = 1
F32 = mybir.dt.float32
I32 = mybir.dt.int32
U32 = mybir.dt.uint32
U8 = mybir.dt.uint8
AF = mybir.ActivationFunctionType
OP = mybir.AluOpType

_BUILD_CACHE = {}


def build_kernel(steps: int):
    if steps in _BUILD_CACHE:
        return _BUILD_CACHE[steps]
    nc = bacc.Bacc("TRN2", target_bir_lowering=False, debug=False,
                   enable_asserts=False, num_devices=NCORES)

    dt = nc.dram_tensor
    # --- per-core inputs (weights pre-sliced/transposed on host) ---
    wz0f_d = dt("wz0f", [P, 8, 512], F32, kind="ExternalInput")
    wz0r_d = dt("wz0r", [P, 8, 512], F32, kind="ExternalInput")
    wz1f_d = dt("wz1f", [P, 16, 512], F32, kind="ExternalInput")
    wz1r_d = dt("wz1r", [P, 16, 512], F32, kind="ExternalInput")
    wfc_d = dt("wfc", [P, 16, 128], F32, kind="ExternalInput")
    b1f_d = dt("b1f", [P, 512], F32, kind="ExternalInput")
    b1r_d = dt("b1r", [P, 512], F32, kind="ExternalInput")
    fcb_d = dt("fcb", [P, 128], F32, kind="ExternalInput")
    eos_d = dt("eos", [P, 128], F32, kind="ExternalInput")
    vbase_d = dt("vbase", [P, 1], F32, kind="ExternalInput")
    ident_d = dt("ident", [P, P], F32, kind="ExternalInput")
    gx0f_d = dt("gx0f", [1024, 512], F32, kind="ExternalInput")  # stays in DRAM
    gx0r_d = dt("gx0r", [1024, 512], F32, kind="ExternalInput")  # stays in DRAM
    h0t_d = dt("h0t", [P, 2, 8, P], F32, kind="ExternalInput")
    h1ft_d = dt("h1ft", [P, 8, P], F32, kind="ExternalInput")
    h1rt_d = dt("h1rt", [P, 8, P], F32, kind="ExternalInput")
    h1b_d = dt("h1b", [P, 2048], F32, kind="ExternalInput")  # initial batch-major A
    feed0_d = dt("feed0", [P, 1], I32, kind="ExternalInput")
    # --- output: this core's vocab slice of softmax(logits) ---
    out_d = dt("out", [P, steps, 128], F32, kind="ExternalOutput")

    with tile.TileContext(nc) as tc:
        from contextlib import ExitStack
        es = ExitStack()
        W = es.enter_context(tc.tile_pool(name="wpool", bufs=1))
        S = es.enter_context(tc.tile_pool(name="state", bufs=1))
        D2 = es.enter_context(tc.tile_pool(name="work", bufs=2))
        D3 = es.enter_context(tc.tile_pool(name="gwork", bufs=2))
        PS0 = es.enter_context(tc.tile_pool(name="psz0", bufs=1, space="PSUM"))
        PS = es.enter_context(tc.tile_pool(name="psz", bufs=1, space="PSUM"))
        PSF = es.enter_context(tc.tile_pool(name="psf", bufs=1, space="PSUM"))
        PST = es.enter_context(tc.tile_pool(name="pst", bufs=2, space="PSUM"))
        DR = es.enter_context(tc.tile_pool(name="dram", bufs=2, space="DRAM"))

        # resident weights
        wz0f = W.tile([P, 8, 512], F32, tag="wz0f", name="wz0f")
        wz0r = W.tile([P, 8, 512], F32, tag="wz0r", name="wz0r")
        wz1f = W.tile([P, 16, 512], F32, tag="wz1f", name="wz1f")
        wz1r = W.tile([P, 16, 512], F32, tag="wz1r", name="wz1r")
        wfc = W.tile([P, 16, 128], F32, tag="wfc", name="wfc")
        b1f = W.tile([P, 512], F32, tag="b1f", name="b1f")
        b1r = W.tile([P, 512], F32, tag="b1r", name="b1r")
        fcb = W.tile([P, 128], F32, tag="fcb", name="fcb")
        eos = W.tile([P, 128], F32, tag="eos", name="eos")
        vbase = W.tile([P, 1], F32, tag="vbase", name="vbase")
        ident = W.tile([P, P], F32, tag="ident", name="ident")
        for t_, d_ in [(wz0f, wz0f_d), (wz0r, wz0r_d), (wz1f, wz1f_d),
                       (wz1r, wz1r_d), (wfc, wfc_d), (b1f, b1f_d), (b1r, b1r_d),
                       (fcb, fcb_d), (eos, eos_d), (vbase, vbase_d), (ident, ident_d)]:
            nc.sync.dma_start(t_[:], d_.ap())

        # state tiles (updated in place across steps)
        cf0 = S.tile([P, P], F32, tag="cf0", name="cf0")
        cr0 = S.tile([P, P], F32, tag="cr0", name="cr0")
        cf1 = S.tile([P, P], F32, tag="cf1", name="cf1")
        cr1 = S.tile([P, P], F32, tag="cr1", name="cr1")
        ended = S.tile([P, 1], U8, tag="ended", name="ended")
        label = S.tile([P, 1], I32, tag="label", name="label")
        for c_ in (cf0, cr0, cf1, cr1):
            nc.vector.memset(c_[:], 0.0)
        nc.vector.memset(ended[:], 0.0)
        nc.sync.dma_start(label[:], feed0_d.ap())

        # h tiles (persistent; rewritten in place each step)
        h0t = S.tile([P, 2, 8, P], F32, tag="h0t", name="h0t")
        h1ft = S.tile([P, 8, P], F32, tag="h1ft", name="h1ft")
        h1rt = S.tile([P, 8, P], F32, tag="h1rt", name="h1rt")
        Ab = S.tile([P, 2048], F32, tag="Ab", name="Ab")
        nc.sync.dma_start(h0t[:], h0t_d.ap())
        nc.sync.dma_start(h1ft[:], h1ft_d.ap())
        nc.sync.dma_start(h1rt[:], h1rt_d.ap())
        nc.sync.dma_start(Ab[:], h1b_d.ap())

        def gates(zb, c, h2, tmp_tag):
            """zb [P,512] pre-activation AP (ifog layout) -> updates c, writes h2 [P,128]."""
            tio = D3.tile([P, 384], F32, tag=tmp_tag + "tio", name=tmp_tag + "tio")
            tg = D3.tile([P, P], F32, tag=tmp_tag + "tg", name=tmp_tag + "tg")
            nc.scalar.activation(tio[:], zb[:, 0:384], AF.Tanh, scale=0.5)
            nc.vector.tensor_scalar(tio[:], tio[:], 0.5, 0.5, op0=OP.mult, op1=OP.add)
            nc.scalar.activation(tg[:], zb[:, 384:512], AF.Tanh)
            m1 = D3.tile([P, P], F32, tag=tmp_tag + "m1", name=tmp_tag + "m1")
            nc.vector.tensor_tensor(m1[:], tio[:, 128:256], c[:], op=OP.mult)  # sig(f)*c
            nc.vector.tensor_tensor(tg[:], tio[:, 0:128], tg[:], op=OP.mult)   # sig(i)*tanh(g)
            nc.vector.tensor_tensor(c[:], m1[:], tg[:], op=OP.add)             # c2
            nc.scalar.activation(m1[:], c[:], AF.Tanh)                         # tanh(c2)
            nc.vector.tensor_tensor(h2[:], tio[:, 256:384], m1[:], op=OP.mult)  # sig(o)*tanh(c2)

        for t in range(steps):
            # ---- layer-0 x contribution: gather Gx0[label] (emb @ W_ih0^T + b0)
            # split by direction so gates0-f can start while the r half lands ----
            xgf = D2.tile([P, 512], F32, tag="xgf", name="xgf")
            xgr = D2.tile([P, 512], F32, tag="xgr", name="xgr")
            nc.gpsimd.indirect_dma_start(
                out=xgf[:], out_offset=None, in_=gx0f_d.ap(),
                in_offset=bass.IndirectOffsetOnAxis(ap=label[:, :1], axis=0),
            )
            nc.gpsimd.indirect_dma_start(
                out=xgr[:], out_offset=None, in_=gx0r_d.ap(),
                in_offset=bass.IndirectOffsetOnAxis(ap=label[:, :1], axis=0),
            )

            # ---- layer-0 z matmuls (h-part); ready at step start ----
            zps0f = PS0.tile([P, 512], F32, tag="zps0f", name="zps0f")
            zps0r = PS0.tile([P, 512], F32, tag="zps0r", name="zps0r")
            for k in range(8):
                nc.tensor.matmul(zps0f[:], h0t[:, 0, k, :], wz0f[:, k, :],
                                 start=(k == 0), stop=(k == 7))
            for k in range(8):
                nc.tensor.matmul(zps0r[:], h0t[:, 1, k, :], wz0r[:, k, :],
                                 start=(k == 0), stop=(k == 7))

            # ---- layer-1 z: h1(t-1) part (inputs ready at step start) ----
            zps1f = PS.tile([P, 512], F32, tag="zps1f", name="zps1f")
            zps1r = PS.tile([P, 512], F32, tag="zps1r", name="zps1r")
            for k in range(8):
                nc.tensor.matmul(zps1f[:], h1ft[:, k, :], wz1f[:, 8 + k, :],
                                 start=(k == 0), stop=False)
            for k in range(8):
                nc.tensor.matmul(zps1r[:], h1rt[:, k, :], wz1r[:, 8 + k, :],
                                 start=(k == 0), stop=False)

            # ---- layer-0 gates (bias+x already in xgf/xgr; add z in place) ----
            h2f0 = D3.tile([P, P], F32, tag="h2f0", name="h2f0")
            h2r0 = D3.tile([P, P], F32, tag="h2r0", name="h2r0")
            nc.vector.tensor_tensor(xgf[:], zps0f[:], xgf[:], op=OP.add)
            gates(xgf[:], cf0, h2f0, "gf")
            nc.vector.tensor_tensor(xgr[:], zps0r[:], xgr[:], op=OP.add)
            gates(xgr[:], cr0, h2r0, "gr")

            # ---- transpose own h0 slices into one staging tile, AG h0 ----
            agh0_in = DR.tile([2, P, P], F32, tag="agh0i", name="agh0i")
            agh0_out = DR.tile([NCORES, 2, P, P], F32, tag="agh0o", name="agh0o")
            st0 = D3.tile([P, 2, P], F32, tag="st0", name="st0")
            for s_, h2_ in ((0, h2f0), (1, h2r0)):
                tp = PST.tile([P, P], F32, tag="tp", name="tp0")
                nc.tensor.transpose(tp[:], h2_[:], ident[:])
                nc.vector.tensor_copy(st0[:, s_, :], tp[:])
                nc.sync.dma_start(agh0_in[s_].rearrange("p b -> p b"), st0[:, s_, :])
            nc.gpsimd.collective_compute(
                "AllGather", OP.bypass, replica_groups=[list(range(NCORES))],
                ins=[agh0_in.opt()], outs=[agh0_out.opt()],
            )
            # pipelined readback: 2-chunk (k-pair) DMAs alternating Sync/Scalar
            # queues so the first z1x matmuls start ~3us after the AG lands
            # (matmul emission order k=0..7 f then r is unchanged -> identical
            # PSUM accumulation order -> bitwise-identical numerics).
            ag0 = agh0_out[:].rearrange("r s p b -> p s r b")
            for s_ in (0, 1):
                for g in range(4):
                    eng = nc.sync if (g % 2 == 0) else nc.scalar
                    eng.dma_start(h0t[:, s_, 2 * g:2 * g + 2, :],
                                  ag0[:, s_, 2 * g:2 * g + 2, :])

            # ---- layer-1 z: h0(t) part; gates-f emitted between the f and r
            # matmul groups so its semaphore wait covers only the f group ----
            hb1 = D3.tile([P, 2, P], F32, tag="hb1", name="hb1")
            zb1f = D3.tile([P, 512], F32, tag="zb1", name="zb1f")
            zb1r = D3.tile([P, 512], F32, tag="zb1", name="zb1r", bufs=2)
            for k in range(8):
                nc.tensor.matmul(zps1f[:], h0t[:, 0, k, :], wz1f[:, k, :],
                                 start=False, stop=(k == 7))
            nc.vector.tensor_tensor(zb1f[:], zps1f[:], b1f[:], op=OP.add)
            gates(zb1f, cf1, hb1[:, 0, :], "gf")
            for k in range(8):
                nc.tensor.matmul(zps1r[:], h0t[:, 1, k, :], wz1r[:, k, :],
                                 start=False, stop=(k == 7))
            nc.vector.tensor_tensor(zb1r[:], zps1r[:], b1r[:], op=OP.add)
            gates(zb1r, cr1, hb1[:, 1, :], "gr")

            # ---- AG-h1 (batch-major, critical path: feeds Ab/fc); f half
            # staged while gates-r still runs ----
            agh1b_in = DR.tile([2, P, P], F32, tag="agh1bi", name="agh1bi")
            agh1b_out = DR.tile([NCORES, 2, P, P], F32, tag="agh1bo", name="agh1bo")
            nc.sync.dma_start(agh1b_in[0], hb1[:, 0, :])
            nc.sync.dma_start(agh1b_in[1], hb1[:, 1, :])
            nc.gpsimd.collective_compute(
                "AllGather", OP.bypass, replica_groups=[list(range(NCORES))],
                ins=[agh1b_in.opt()], outs=[agh1b_out.opt()],
            )
            # A[b, s*1024 + r*128 + fl] = agh1b_out[r, s, b, fl]
            # (4 half-width readbacks split across Sync/Scalar queues)
            for s_ in (0, 1):
                for hh in (0, 1):
                    eng = nc.sync if hh == 0 else nc.scalar
                    eng.dma_start(
                        Ab[:, 1024 * s_ + 512 * hh:1024 * s_ + 512 * (hh + 1)]
                        .rearrange("b (r fl) -> b r fl", r=4),
                        agh1b_out[4 * hh:4 * (hh + 1), s_]
                        .rearrange("r b fl -> b r fl"),
                    )

            # ---- fc: logits[i, v] = sum_q A[:, q::16].T @ wfc[:, q, :] ----
            fcps = PSF.tile([P, 128], F32, tag="fcps", name="fcps")
            Astr = Ab[:].rearrange("p (j s) -> p s j", s=16)
            for q in range(16):
                nc.tensor.matmul(fcps[:], Astr[:, q, :], wfc[:, q, :],
                                 start=(q == 0), stop=(q == 15))

            # ---- epilogue: mask, stats, exp ----
            lg = D3.tile([P, 128], F32, tag="lg", name="lg")
            nc.vector.tensor_tensor(lg[:], fcps[:], fcb[:], op=OP.add)
            nc.vector.copy_predicated(lg[:], ended[:, :1].to_broadcast([P, 128]), eos[:])
            mv = D3.tile([P, 8], F32, tag="mv", name="mv")
            mi = D3.tile([P, 8], U32, tag="mi", name="mi")
            nc.vector.max_with_indices(mv[:], mi[:], lg[:])
            ex = D3.tile([P, 128], F32, tag="ex", name="ex")
            sm = D3.tile([P, 1], F32, tag="sm", name="sm")
            nc.scalar.activation(ex[:], lg[:], AF.Exp, accum_out=sm[:])
            stats = D3.tile([P, 4], F32, tag="stats", name="stats")
            nc.vector.tensor_copy(stats[:, 0:1], mv[:, 0:1])
            nc.vector.tensor_copy(stats[:, 1:2], mi[:, 0:1])  # uint32 -> f32
            nc.vector.tensor_tensor(stats[:, 1:2], stats[:, 1:2], vbase[:], op=OP.add)
            nc.vector.tensor_copy(stats[:, 2:3], sm[:])
            nc.vector.tensor_copy(stats[:, 3:4], sm[:])

            ags_in = DR.tile([P, 4], F32, tag="agsi", name="agsi")
            ags_out = DR.tile([NCORES, P, 4], F32, tag="agso", name="agso")
            nc.sync.dma_start(ags_in[:], stats[:])
            nc.gpsimd.collective_compute(
                "AllGather", OP.bypass, replica_groups=[list(range(NCORES))],
                ins=[ags_in.opt()], outs=[ags_out.opt()],
            )
            sa = D3.tile([P, NCORES, 4], F32, tag="sa", name="sa")
            nc.sync.dma_start(sa[:], ags_out[:].rearrange("r p s -> p r s"))

            # ---- combine: tournament argmax in place on sa (strict-gt => first wins) ----
            gt = D3.tile([P, 4], U8, tag="gt", name="gt")
            for lvl, b in ((0, 2), (1, 4), (2, 8)):
                n = NCORES // b
                sv = sa[:].rearrange("p (a b) s -> p a b s", b=b)
                lo_v, hi_v = sv[:, :, 0, 0:1], sv[:, :, b // 2, 0:1]
                lo_i, hi_i = sv[:, :, 0, 1:2], sv[:, :, b // 2, 1:2]
                g = gt[:, 0:n]
                nc.vector.tensor_tensor(g, hi_v, lo_v, op=OP.is_gt)
                nc.vector.copy_predicated(lo_v, g, hi_v)
                nc.vector.copy_predicated(lo_i, g, hi_i)
            i_ = sa[:, 0, 1:2]
            # ---- label + ended update first (they gate the next step's gather) ----
            nc.vector.tensor_copy(label[:], i_)  # f32 -> int32
            eq = D3.tile([P, 1], U8, tag="eq", name="eq")
            nc.vector.tensor_scalar(eq[:], i_, float(EOS), None, op0=OP.is_equal)
            nc.vector.tensor_tensor(ended[:], ended[:], eq[:], op=OP.max)
            # ---- softmax output (off the recurrence; out write on Scalar queue) ----
            gs = D3.tile([P, 1], F32, tag="gs", name="gs")
            nc.vector.tensor_reduce(gs[:], sa[:, :, 2:3], axis=mybir.AxisListType.XY, op=OP.add)
            nc.vector.reciprocal(gs[:], gs[:])
            ob = D3.tile([P, 128], F32, tag="ob", name="ob")
            nc.vector.tensor_scalar(ob[:], ex[:], gs[:, :1], None, op0=OP.mult)
            nc.scalar.dma_start(out_d.ap()[:, t, :], ob[:])

            # ---- derive next step's feature-major h1 from Ab (PE transposes) ----
            for r in range(8):
                tpa = PST.tile([P, P], F32, tag="tp", name="tpa_f")
                nc.tensor.transpose(tpa[:], Ab[:, 128 * r:128 * (r + 1)], ident[:])
                nc.vector.tensor_copy(h1ft[:, r, :], tpa[:])
            for r in range(8):
                tpa = PST.tile([P, P], F32, tag="tp", name="tpa_r")
                nc.tensor.transpose(tpa[:], Ab[:, 1024 + 128 * r:1024 + 128 * (r + 1)], ident[:])
                nc.vector.tensor_copy(h1rt[:, r, :], tpa[:])

        es.close()

    nc.compile()
    _BUILD_CACHE[steps] = nc
    return nc


def _pack_inputs(yy_pad, h_t, h_t_rev, x_lens, emb, W_ih, W_hh, b_ih, b_hh,
                 W_ih_rev, W_hh_rev, b_ih_rev, b_hh_rev, c0, c0_rev, fc_W, fc_b,
                 steps):
    f32 = np.float32
    ar = np.arange(128)
    in_maps = []
    # feature-major initial h chunks: [p, k, b] = h[b, 128k+p]
    def tfm(hm):
        return np.ascontiguousarray(
            hm.T.reshape(8, 128, 128).transpose(1, 0, 2)).astype(f32)

    emb64 = emb.astype(np.float64)
    gx_f = emb64 @ W_ih[0].astype(np.float64).T + (b_ih[0] + b_hh[0]).astype(np.float64)
    gx_r = emb64 @ W_ih_rev[0].astype(np.float64).T + (b_ih_rev[0] + b_hh_rev[0]).astype(np.float64)
    Wcat1f = np.concatenate([W_ih[1], W_hh[1]], axis=1)
    Wcat1r = np.concatenate([W_ih_rev[1], W_hh_rev[1]], axis=1)
    A_init = np.concatenate([h_t[1], h_t_rev[1]], axis=1).astype(f32)
    h0t_init = np.ascontiguousarray(
        np.stack([tfm(h_t[0]), tfm(h_t_rev[0])], axis=1))

    for d in range(NCORES):
        cols = np.concatenate([1024 * 0 + 128 * d + ar, 1024 * 1 + 128 * d + ar,
                               1024 * 3 + 128 * d + ar, 1024 * 2 + 128 * d + ar])
        wz0f = np.ascontiguousarray(
            W_hh[0][cols, :].T.reshape(8, 128, 512).transpose(1, 0, 2)).astype(f32)
        wz0r = np.ascontiguousarray(
            W_hh_rev[0][cols, :].T.reshape(8, 128, 512).transpose(1, 0, 2)).astype(f32)
        wz1f = np.ascontiguousarray(
            Wcat1f[cols, :].T.reshape(16, 128, 512).transpose(1, 0, 2)).astype(f32)
        wz1r = np.ascontiguousarray(
            Wcat1r[cols, :].T.reshape(16, 128, 512).transpose(1, 0, 2)).astype(f32)
        # wfc[c, q, v] = fc_W[128d+v, 128q+c]
        wfc = np.ascontiguousarray(
            fc_W[128 * d:128 * (d + 1), :].reshape(128, 16, 128).transpose(2, 1, 0)).astype(f32)
        gx0f = np.ascontiguousarray(gx_f[:, cols]).astype(f32)
        gx0r = np.ascontiguousarray(gx_r[:, cols]).astype(f32)
        b1f = np.broadcast_to((b_ih[1] + b_hh[1])[cols], (P, 512)).astype(f32)
        b1r = np.broadcast_to((b_ih_rev[1] + b_hh_rev[1])[cols], (P, 512)).astype(f32)
        fcb = np.broadcast_to(fc_b[128 * d:128 * (d + 1)], (P, 128)).astype(f32)
        eos_sl = np.zeros(128, f32)
        if d == 0:
            eos_sl[EOS] = 1.0
        eos_t = np.broadcast_to(eos_sl, (P, 128)).copy()
        in_maps.append(dict(
            wz0f=wz0f, wz0r=wz0r, wz1f=wz1f, wz1r=wz1r, wfc=wfc,
            b1f=np.ascontiguousarray(b1f), b1r=np.ascontiguousarray(b1r),
            fcb=np.ascontiguousarray(fcb), eos=eos_t,
            vbase=np.full((P, 1), 128.0 * d, f32),
            ident=np.eye(P, dtype=f32),
            gx0f=gx0f, gx0r=gx0r,
            h0t=h0t_init,
            h1ft=tfm(h_t[1]), h1rt=tfm(h_t_rev[1]),
            h1b=A_init,
            feed0=yy_pad[:, 0:1].astype(np.int32),
        ))
    return in_maps


def kernel(yy_pad, h_t, h_t_rev, x_lens, emb, W_ih, W_hh, b_ih, b_hh,
           W_ih_rev, W_hh_rev, b_ih_rev, b_hh_rev, c0, c0_rev, fc_W, fc_b,
           steps=T_FULL, trace=False):
    args = [np.asarray(a) for a in
            (yy_pad, h_t, h_t_rev, x_lens, emb, W_ih, W_hh, b_ih, b_hh,
             W_ih_rev, W_hh_rev, b_ih_rev, b_hh_rev, c0, c0_rev, fc_W, fc_b)]
    nc = build_kernel(steps)
    in_maps = _pack_inputs(*args, steps)
    res = run_bass_kernel_spmd(nc, in_maps, core_ids=list(range(NCORES)),
                               trace=trace)
    out = np.concatenate([res.results[d]["out"] for d in range(NCORES)], axis=2)
    kernel.last_exec_time_ns = res.exec_time_ns
    return out.astype(np.float32)
